# revision 29
# baseline (speedup 1.0000x reference)
"""BertSelfAttention (RoPE, non-causal) Trainium2 kernel, 8-core SPMD.

Problem: hidden_states [4, 2048, 1024], H=16 heads x 64 dim, Wq/Wk/Wv [1024,1024]
         out = softmax((rope(q) @ rope(k).T)/8) @ v   -> [4, 2048, 1024]

Sharding: 8 cores = (batch b in 0..3) x (head-group g in 0..1).
Each core handles batch b, heads g*8..g*8+7 (output columns g*512..(g+1)*512).

Per-core layout strategy (host pre-transposes; no on-chip transposes):
  xT  [D=1024, S=2048] fp16      (hidden_states[b].T)
  wT  [D=1024, E=512]  fp16      (W[g*512:(g+1)*512, :].T for q/k/v)
  QT,KT = (x @ W.T).T computed directly as [E, S] via lhsT=wT, rhs=xT
  V     = x @ Wv.T computed as [S, E] via lhsT=xT, rhs=wvT
  rope on QT/KT in [e, s] layout (partition-sliced DVE ops from PSUM)
  scoresT[k, q] = lhsT(KT slice).T @ rhs(QT slice)  -- 2 heads row-tiled (K=64)
  expT = exp(scoresT/8 + c): ACT table exp for most tiles; for kt in
         {13,15} of blocks 1..15 a Schraudolph bit-trick on DVE
         (int16(x*1024/ln2 + const) reinterpreted as fp16, ~1.8% RMS)
  ctxT[hd, q] += V_slice.T @ expT  -- 2 heads col-tiled; denom via ones-matmuls

Schedule (v3): single flat software-pipelined loop over all 256 (block, kt)
iterations; scores run one iteration ahead of ctx; V projection interleaved
into block 0; Q/K projections for later pairs spread as 2 filler chunks per
block; ctx+denominator staged once per (head, q-chunk) and DMA'd as one
[65, 512] transfer.

v4: (a) head-dim rows permuted (ROPE_PERM) so rotate-half is an
intra-quadrant 16-row swap -> rope = stream_shuffle + 2 muls + add on the
fp16 2x DVE path instead of 4 partition-sliced psum muls; (b) prologue cut
to K0+Q0 with ACT psum->fp16 assists, K1-3/Q1 moved into block 0 as
fillers; (c) V-tile psum casts moved to ACT; (d) more Schraudolph slots
(3/16 mid, 6/16 in tail blocks where DVE idles).
"""

import os
import sys
import types

import numpy as np

import concourse.bass as bass
import concourse.tile as tile
from concourse import mybir
from concourse.vector_clock import ScopedClock

B, S, D, H = 4, 2048, 1024, 16
HD = 64          # head dim
E = 512          # output dims per core (8 heads)
N_CORES = 8
QC = 512         # q chunk (moving free dim)
KT_TILE = 128    # k tile (stationary cols / psum partitions)
N_QC = S // QC           # 4
N_KT = S // KT_TILE      # 16
FP16 = mybir.dt.float16
FP32 = mybir.dt.float32

# Schraudolph exp-in-fp16-bits: bits = x*(1024/ln2) + (15*1024 + sigma), with
# x = scores*0.125 - 2 folded in: bits = scores*SCH_A + SCH_B, sigma = -59.
SCH_A = 184.6648378
# B = 15360 - 59(sigma) + 453: value ~= exp(s/8)*2^(453/1024) = exp(s/8+0.30663),
# keeping int16 bits positive for scores down to -85 (data min is -76).
SCH_B = 15360.0 - 59.0 + 453.0
ACT_BIAS = 0.3066336   # ACT exp bias matching the Schraudolph 2^(453/1024) scale
# kt slots per block computed on DVE (Schraudolph) for filler-rich blocks.
# From MID2_BLK on, every iteration instead splits exp across BOTH engines
# (h0 on ACT, h1 on DVE) so the scores psum frees after ~690ns instead of
# a 1.2-1.4us serial read -- the psum WAR is the tail's critical path.
SCHRAU_KT = (5, 7, 11, 13, 15)                   # blocks 1..MID2_BLK-1
MID2_BLK = 10
TAIL_BLK = 12

# Head-dim row layout: each head's 64 dims stored as [0..15, 32..47] in its
# first 32-partition quadrant and [16..31, 48..63] in its second, so
# rotate-half (i <-> i+32) becomes an intra-quadrant 16-row swap that DVE
# stream_shuffle can do in one pass. Scores are invariant to any shared
# row permutation of Q/K/cos/sin.
ROPE_PERM = ([i for i in range(16)] + [32 + i for i in range(16)]
             + [16 + i for i in range(16)] + [48 + i for i in range(16)])
SHUF_MASK = [i + 16 if i < 16 else i - 16 for i in range(32)]

# ---------------------------------------------------------------------------
# Environment fixups (old nix walrus: max 1 sync wait per instruction; and the
# axon NTFF profile hook module is missing from the image's antenv).
# ---------------------------------------------------------------------------

_PATCHED = False


def _patched_drain_and_barrier(self, tick_clock, wait_clock):
    nc = self.nc
    nops = []
    for _ in range(24):
        nop = mybir.InstNoOp(
            name=nc.get_next_instruction_name(),
            text_hint="wait_split",
            bass_nofuse=True,
            engine=mybir.EngineType.SP,
        )
        nc.add_instruction(nop)
        nops.append(nop)
    drain_inst = nc.sync.drain()
    wait_clock.add_sem_waits(
        drain_inst.ins, ScopedClock({None: tick_clock.global_clock})
    )
    si = drain_inst.ins.sync_info
    if si is not None and si.on_wait and len(si.on_wait) > 1:
        extras = list(si.on_wait[1:])
        si.on_wait = si.on_wait[:1]
        assert len(extras) <= len(nops)
        for nop, w in zip(nops, extras):
            nop.sync_info = mybir.SyncInfo(on_wait=[w], on_update=[])

    nc.all_engine_barrier()
    assert self.sems is not None
    popped = nc._tile_sem_poison_stack.pop()
    assert popped is self._sem_poison
    nc.clear_and_free_semaphores(list(self.sems.allocated().values()))
    nc.all_engine_barrier()


_ORIG_POSTORDER = tile.postorder_instruction_blocks
_SPLIT_COUNTER = [0]


def _split_excess_waits(instructions):
    """Old walrus encodes at most 1 sync wait per instruction (2 for
    EventSemaphore). Hoist extras onto preceding same-engine NoOps — the
    engine is in-order, so gating semantics are identical."""
    for bb_name, insts in instructions.items():
        out = []
        for inst in insts:
            si = getattr(inst, "sync_info", None)
            waits = list(si.on_wait) if (si is not None and si.on_wait) else []
            cap = 2 if isinstance(inst, mybir.InstEventSemaphore) else 1
            if len(waits) > cap:
                eng = inst.engine
                assert eng != mybir.EngineType.Unassigned, (
                    f"multi-wait inst {inst.name} has no engine"
                )
                si.on_wait = waits[:cap]
                for w in waits[cap:]:
                    _SPLIT_COUNTER[0] += 1
                    nop = mybir.InstNoOp(
                        name=f"waitsplit_{_SPLIT_COUNTER[0]}",
                        text_hint="wait_split",
                        bass_nofuse=True,
                        engine=eng,
                        sync_info=mybir.SyncInfo(on_wait=[w], on_update=[]),
                    )
                    out.append(nop)
            out.append(inst)
        instructions[bb_name] = out


def _patched_postorder(instructions, start_bb, output):
    if not output:  # only at the top-level invocation
        _split_excess_waits(instructions)
    return _ORIG_POSTORDER(instructions, start_bb, output)


def _install_fixups():
    global _PATCHED
    if not _PATCHED:
        tile.TileContext._drain_and_barrier = _patched_drain_and_barrier
        tile.postorder_instruction_blocks = _patched_postorder
        _PATCHED = True
    if "antenv.axon_hooks" not in sys.modules:
        mod = types.ModuleType("antenv.axon_hooks")
        _state = {"hook": None}
        mod.set_axon_ntff_profile_hook = lambda h: _state.__setitem__("hook", h)
        mod.get_axon_ntff_profile_hook = lambda: _state["hook"]
        sys.modules["antenv.axon_hooks"] = mod
        try:
            from trn_agent_boot.trn_boot import _ntff_profile_via_ctypes

            mod.set_axon_ntff_profile_hook(
                _ntff_profile_via_ctypes("/opt/axon/libaxon_pjrt.so")
            )
        except Exception:
            pass


# ---------------------------------------------------------------------------
# Kernel build
# ---------------------------------------------------------------------------


def build_nc():
    _install_fixups()
    nc = bass.Bass(trn_type="TRN2", target_bir_lowering=False, debug=False)

    xt_d = nc.dram_tensor("xt", [D, S], FP16, kind="ExternalInput").ap()
    wqt_d = nc.dram_tensor("wqt", [D, E], FP16, kind="ExternalInput").ap()
    wkt_d = nc.dram_tensor("wkt", [D, E], FP16, kind="ExternalInput").ap()
    wvt_d = nc.dram_tensor("wvt", [D, E], FP16, kind="ExternalInput").ap()
    cos2_d = nc.dram_tensor("cos2", [128, S], FP16, kind="ExternalInput").ap()
    sinn2_d = nc.dram_tensor("sinn2", [128, S], FP16, kind="ExternalInput").ap()
    # merged ctx+denominator output: [head, 64 ctx rows + 1 denom row, S]
    o65_d = nc.dram_tensor("o65_out", [8, N_QC, 128, 4 * 65], FP32,
                           kind="ExternalOutput").ap()

    with tile.TileContext(nc) as tc:
        import contextlib

        ctx = contextlib.ExitStack()
        with ctx:
            p_xt = ctx.enter_context(tc.tile_pool(name="xt", bufs=8))
            p_w = ctx.enter_context(tc.tile_pool(name="w", bufs=24))
            p_trig = ctx.enter_context(tc.tile_pool(name="trig", bufs=2))
            p_qk = ctx.enter_context(tc.tile_pool(name="qk", bufs=8))
            p_v = ctx.enter_context(tc.tile_pool(name="v", bufs=16))
            p_exp = ctx.enter_context(tc.tile_pool(name="exp", bufs=6))
            p_tmp = ctx.enter_context(tc.tile_pool(name="tmp", bufs=8))
            p_one = ctx.enter_context(tc.tile_pool(name="one", bufs=1))
            p_stage = ctx.enter_context(tc.tile_pool(name="stage", bufs=4))
            ps_sc = ctx.enter_context(
                tc.tile_pool(name="ps_sc", bufs=2, space="PSUM"))
            ps_cx = ctx.enter_context(
                tc.tile_pool(name="ps_cx", bufs=2, space="PSUM"))
            ps_f = ctx.enter_context(
                tc.tile_pool(name="ps_f", bufs=2, space="PSUM"))

            # ---- loads ----
            # xt on sync HWDGE; wk+wv on gpsimd SWDGE; trig+wq on ACT HWDGE
            # (ordered by first use: proj0 needs wk/trig/wq early, wv later).
            xt_tiles = []
            for dt_i in range(8):
                t = p_xt.tile([128, S], FP16, tag="xt")
                nc.sync.dma_start(t[:], xt_d[dt_i * 128:(dt_i + 1) * 128, :])
                xt_tiles.append(t)

            w_tiles = {}

            def load_w(nm, dram, eng):
                tl = []
                for dt_i in range(8):
                    t = p_w.tile([128, E], FP16, tag="w", name=f"w{nm}{dt_i}")
                    eng.dma_start(t[:], dram[dt_i * 128:(dt_i + 1) * 128, :])
                    tl.append(t)
                w_tiles[nm] = tl

            # weights on gpsimd SWDGE (spreads transfers across DMA rings);
            # trig on the scalar HWDGE queue so it loads in parallel with wk
            load_w("k", wkt_d, nc.gpsimd)
            cos2 = p_trig.tile([128, S], FP16, tag="trig")
            nc.scalar.dma_start(cos2[:], cos2_d[:])
            sinn2 = p_trig.tile([128, S], FP16, tag="trig")
            nc.scalar.dma_start(sinn2[:], sinn2_d[:])
            load_w("q", wqt_d, nc.gpsimd)
            load_w("v", wvt_d, nc.gpsimd)
            expbias = p_one.tile([128, 1], FP32)
            nc.gpsimd.memset(expbias[:], ACT_BIAS)

            qt_tiles = [None] * 4
            kt_tiles = [None] * 4
            v_tiles = [None] * 16

            def proj_start(p, kind, sc):
                """Allocate the psum chunk for proj (p, kind, sc)."""
                if kind == "k":
                    if kt_tiles[p] is None:
                        kt_tiles[p] = p_qk.tile([128, S], FP16, tag="qk",
                                                name=f"ktt{p}")
                    out_tile = kt_tiles[p]
                else:
                    if qt_tiles[p] is None:
                        qt_tiles[p] = p_qk.tile([128, S], FP16, tag="qk",
                                                name=f"qtt{p}")
                    out_tile = qt_tiles[p]
                qp = ps_f.tile([128, QC], FP32, tag="psf",
                               name=f"qp_{kind}{p}_{sc}")
                return (p, kind, sc, qp, out_tile)

            def proj_mm(st8, dt_lo, dt_hi):
                p, kind, sc, qp, _ = st8
                wt = w_tiles[kind]
                for dt_i in range(dt_lo, dt_hi):
                    nc.tensor.matmul(
                        qp[:],
                        lhsT=wt[dt_i][:, p * 128:(p + 1) * 128],
                        rhs=xt_tiles[dt_i][:, sc * QC:(sc + 1) * QC],
                        start=(dt_i == 0),
                        stop=(dt_i == 7),
                    )

            def proj_rope(st8, act_assist=False):
                # head dims are row-permuted (ROPE_PERM) so rotate-half is an
                # intra-quadrant 16-row swap: one DVE stream_shuffle replaces
                # the 4 partition-sliced muls.
                p, kind, sc, qp, out_tile = st8
                cs = slice(sc * QC, (sc + 1) * QC)
                if act_assist:
                    # prologue: ACT (idle) casts psum->fp16 so the DVE ops
                    # run on the 2x 16-bit path and the psum buf frees early
                    src = p_tmp.tile([128, QC], FP16)
                    nc.scalar.copy(src[:], qp[:])
                    qsw = p_tmp.tile([128, QC], FP16)
                else:
                    # shuffle can't cast, so psum fp32 stays fp32 here
                    src = qp
                    qsw = p_tmp.tile([128, QC], FP32)
                nc.vector.stream_shuffle(qsw[:], src[:], SHUF_MASK)
                tmp = p_tmp.tile([128, QC], FP16)
                nc.vector.tensor_mul(tmp[:], qsw[:], sinn2[:, cs])
                tmp2 = p_tmp.tile([128, QC], FP16)
                nc.vector.tensor_mul(tmp2[:], src[:], cos2[:, cs])
                nc.vector.tensor_add(out_tile[:, cs], tmp[:], tmp2[:])

            def emit_proj_chunk(p, kind, sc, act_assist=False):
                st8 = proj_start(p, kind, sc)
                proj_mm(st8, 0, 8)
                proj_rope(st8, act_assist)

            def emit_v_chunk(st):
                vp = ps_f.tile([128, E], FP32, tag="psf", name=f"vp{st}")
                for dt_i in range(8):
                    nc.tensor.matmul(
                        vp[:],
                        lhsT=xt_tiles[dt_i][:, st * 128:(st + 1) * 128],
                        rhs=w_tiles["v"][dt_i][:],
                        start=(dt_i == 0),
                        stop=(dt_i == 7),
                    )
                vt = p_v.tile([128, 8 * 65], FP16, tag="v", name=f"vt{st}")
                vt_r = vt.rearrange("p (h c) -> p h c", h=8)
                nc.vector.memset(vt_r[:, :, 64:65], 1.0)
                # psum->fp16 cast on ACT (block 0's DVE is rope-loaded)
                nc.scalar.copy(
                    vt_r[:, :, 0:64],
                    vp.rearrange("p (h c) -> p h c", h=8),
                )
                v_tiles[st] = vt

            # pair-0 prologue: only K sc0 + Q qc0 before the main loop
            # (unblocks scores kt 0-3); K sc1-3 and Q qc1 move into block 0
            # as fillers so the PE isn't gated on 5 serial DVE ropes.
            emit_proj_chunk(0, "k", 0, act_assist=True)
            emit_proj_chunk(0, "q", 0, act_assist=True)

            # proj filler chunks at (block, kt) slots.
            # Block 0 completes pair 0 inline (ACT-assisted ropes); all
            # remaining chunks run as active fillers (2 MMs/g) in blocks
            # 1-9 only, so blocks 10-15 stay rope-free and the DVE can
            # carry exp halves there without queue delays.
            filler = {}
            filler[(0, 0)] = (0, "k", 1)
            filler[(0, 4)] = (0, "k", 2)
            filler[(0, 8)] = (0, "k", 3)
            filler[(0, 12)] = (0, "q", 1)
            filler[(0, 14)] = (1, "k", 0)
            _SEQ = [(0, "q", 2), (0, "q", 3),
                    (1, "k", 1), (1, "k", 2), (1, "k", 3), (1, "q", 0),
                    (1, "q", 1), (1, "q", 2), (1, "q", 3),
                    (2, "k", 0), (2, "k", 1), (2, "k", 2), (2, "k", 3),
                    (2, "q", 0), (2, "q", 1), (2, "q", 2), (2, "q", 3),
                    (3, "k", 0), (3, "k", 1), (3, "k", 2), (3, "k", 3),
                    (3, "q", 0), (3, "q", 1), (3, "q", 2), (3, "q", 3)]
            _SLOTS = [(b, kt) for b in range(1, 10) for kt in (0, 5, 10)]
            for _sl, _ch in zip(_SLOTS, _SEQ):
                filler[_sl] = _ch

            def emit_scores(g, name):
                blk, kt = g // N_KT, g % N_KT
                p, qc = blk // N_QC, blk % N_QC
                qs = slice(qc * QC, (qc + 1) * QC)
                ks = slice(kt * KT_TILE, (kt + 1) * KT_TILE)
                sc_ps = ps_sc.tile([128, 2 * QC], FP32, tag="pssc", name=name)
                for j in range(2):
                    rs = slice(j * 64, (j + 1) * 64)
                    nc.tensor.matmul(
                        sc_ps[:, j * QC:(j + 1) * QC],
                        lhsT=kt_tiles[p][rs, ks],
                        rhs=qt_tiles[p][rs, qs],
                        start=True,
                        stop=True,
                    )
                return sc_ps

            schrau = {g for g in range(256)
                      if 1 <= g // N_KT < MID2_BLK
                      and g % N_KT in SCHRAU_KT}

            sc_tiles = {0: emit_scores(0, "scps0")}
            emit_v_chunk(0)
            emit_v_chunk(1)

            ex_pre = {}
            ctx_ps = None
            active_fillers = []
            for g in range(256):
                blk, kt = g // N_KT, g % N_KT
                p, qc = blk // N_QC, blk % N_QC
                qs = slice(qc * QC, (qc + 1) * QC)
                if kt == 0:
                    # full-bank tiles; only the (m=0, kt=0) matmul uses
                    # start=True -- it clears the whole bank (hw semantics),
                    # so the other 3 packed chains must accumulate onto it
                    ctx_ps = [
                        ps_cx.tile([128, QC], FP32, tag="pscx",
                                   name=f"ctxps{blk}_{_j}")
                        for _j in range(2)
                    ]
                # exp tile for this iteration (pre-emitted on DVE for
                # Schraudolph slots, else ACT table exp now)
                if g in ex_pre:
                    ex = ex_pre.pop(g)
                else:
                    ex = p_exp.tile([128, 2 * QC], FP16, tag="exp")
                    nc.scalar.activation(
                        ex[:], sc_tiles.pop(g)[:],
                        mybir.ActivationFunctionType.Exp,
                        scale=0.125, bias=expbias[:],
                    )
                # scores one iteration ahead
                if g + 1 < 256:
                    sc_tiles[g + 1] = emit_scores(g + 1, f"scps{g + 1}")
                    if (g + 1) // N_KT >= MID2_BLK:
                        ex2 = p_exp.tile([128, 2 * QC], FP16, tag="exp")
                        scn = sc_tiles.pop(g + 1)
                        nc.scalar.activation(
                            ex2[:, 0:QC], scn[:, 0:QC],
                            mybir.ActivationFunctionType.Exp,
                            scale=0.125, bias=expbias[:],
                        )
                        nc.vector.tensor_scalar(
                            ex2[:, QC:2 * QC].bitcast(mybir.dt.int16),
                            scn[:, QC:2 * QC],
                            SCH_A, SCH_B,
                            mybir.AluOpType.mult, mybir.AluOpType.add,
                        )
                        ex_pre[g + 1] = ex2
                    elif g + 1 in schrau:
                        ex2 = p_exp.tile([128, 2 * QC], FP16, tag="exp")
                        nc.vector.tensor_scalar(
                            ex2[:].bitcast(mybir.dt.int16),
                            sc_tiles.pop(g + 1)[:],
                            SCH_A, SCH_B,
                            mybir.AluOpType.mult, mybir.AluOpType.add,
                        )
                        ex_pre[g + 1] = ex2
                # V projection rides inside block 0 (all 16 chunks needed
                # by its ctx accumulation; two pre-emitted before the loop)
                if blk == 0 and kt <= 13:
                    emit_v_chunk(kt + 2)
                if (blk, kt) in filler:
                    if blk == 0:
                        emit_proj_chunk(*filler[(blk, kt)], act_assist=True)
                    else:
                        active_fillers.append(
                            [proj_start(*filler[(blk, kt)]), 0])
                for af in active_fillers:
                    st8, done = af
                    proj_mm(st8, done, done + 2)
                    af[1] += 2
                    if af[1] == 8:
                        proj_rope(st8)
                active_fillers = [af for af in active_fillers if af[1] < 8]
                for j in range(2):
                    h_loc = 2 * p + j
                    for m in range(4):
                        nc.tensor.matmul(
                            ctx_ps[j][:, m * 65:(m + 1) * 65],
                            lhsT=ex[:, j * QC + m * 128:j * QC + (m + 1) * 128],
                            rhs=v_tiles[kt][:, h_loc * 65:(h_loc + 1) * 65],
                            start=(kt == 0 and m == 0),
                            stop=(kt == N_KT - 1),
                            skip_group_check=True,
                        )
                if kt == N_KT - 1:
                    for j in range(2):
                        h_loc = 2 * p + j
                        st = p_stage.tile([128, 4 * 65], FP32, tag="stage")
                        nc.vector.tensor_copy(st[:], ctx_ps[j][:, 0:260])
                        nc.sync.dma_start(o65_d[h_loc, qc], st[:])
    return nc


_NC_CACHE = None


def _get_nc():
    global _NC_CACHE
    if _NC_CACHE is None:
        _NC_CACHE = build_nc()
    return _NC_CACHE


# ---------------------------------------------------------------------------
# Host-side shard / gather
# ---------------------------------------------------------------------------

LAST_EXEC_TIME_NS = None


def kernel(hidden_states, attention_mask, rope_cos, rope_sin, Wq, Wk, Wv):
    """Full inputs -> full output [4, 2048, 1024] float32."""
    global LAST_EXEC_TIME_NS
    del attention_mask  # module sets it to None in forward

    from concourse.bass_utils import run_bass_kernel_spmd

    hidden_states = np.asarray(hidden_states)
    rope_cos = np.asarray(rope_cos, dtype=np.float32)
    rope_sin = np.asarray(rope_sin, dtype=np.float32)
    Wq, Wk, Wv = (np.asarray(w) for w in (Wq, Wk, Wv))

    # trig tables in [e, s] layout, duplicated across the 2 heads of a tile,
    # sin sign-folded for rotate-half, rows permuted by ROPE_PERM; fp16 for
    # 2x DVE rope
    perm = np.asarray(ROPE_PERM)
    cosT = rope_cos.T.astype(np.float32)[perm]      # [64, S]
    sinT = rope_sin.T.astype(np.float32)
    sinN = np.concatenate([-sinT[:32], sinT[32:]], axis=0)[perm]  # [64, S]
    cos2 = np.ascontiguousarray(
        np.concatenate([cosT, cosT], axis=0)).astype(np.float16)   # [128,S]
    sinn2 = np.ascontiguousarray(
        np.concatenate([sinN, sinN], axis=0)).astype(np.float16)   # [128,S]
    # Wq/Wk output dims permuted per head to match (V / output unpermuted)
    perm_e = np.concatenate([h * 64 + perm for h in range(H)])     # [D]

    in_maps = []
    for c in range(N_CORES):
        b, g = c // 2, c % 2
        xT = np.ascontiguousarray(hidden_states[b].T).astype(np.float16)
        sl = slice(g * E, (g + 1) * E)
        Wq_p = Wq[perm_e][sl, :]
        Wk_p = Wk[perm_e][sl, :]
        in_maps.append({
            "xt": xT,
            "wqt": np.ascontiguousarray(Wq_p.T).astype(np.float16),
            "wkt": np.ascontiguousarray(Wk_p.T).astype(np.float16),
            "wvt": np.ascontiguousarray(Wv[sl, :].T).astype(np.float16),
            "cos2": cos2,
            "sinn2": sinn2,
        })

    nc = _get_nc()
    trace = bool(os.environ.get("BERT_KERNEL_TRACE"))
    res = run_bass_kernel_spmd(nc, in_maps, list(range(N_CORES)), trace=trace)
    LAST_EXEC_TIME_NS = res.exec_time_ns

    out = np.empty((B, S, D), dtype=np.float32)
    for c in range(N_CORES):
        b, g = c // 2, c % 2
        o65 = res.results[c]["o65_out"]   # [8, N_QC, 128, 4*65]
        oq = o65.reshape(8, N_QC, 128, 4, 65).transpose(0, 1, 3, 2, 4)
        oq = np.ascontiguousarray(oq).reshape(8, S, 65)  # q-major per head
        for hh in range(8):
            col = g * E + hh * 64
            out[b, :, col:col + 64] = oq[hh, :, 0:64] / oq[hh, :, 64][:, None]
    return out



# revision 30
# speedup vs baseline: 1.0390x; 1.0390x over previous
"""BertSelfAttention (RoPE, non-causal) Trainium2 kernel, 8-core SPMD.

Problem: hidden_states [4, 2048, 1024], H=16 heads x 64 dim, Wq/Wk/Wv [1024,1024]
         out = softmax((rope(q) @ rope(k).T)/8) @ v   -> [4, 2048, 1024]

Sharding: 8 cores = (batch b in 0..3) x (head-group g in 0..1).
Each core handles batch b, heads g*8..g*8+7 (output columns g*512..(g+1)*512).

Per-core layout strategy (host pre-transposes; no on-chip transposes):
  xT  [D=1024, S=2048] fp16      (hidden_states[b].T)
  wT  [D=1024, E=512]  fp16      (W[g*512:(g+1)*512, :].T for q/k/v)
  QT,KT = (x @ W.T).T computed directly as [E, S] via lhsT=wT, rhs=xT
  V     = x @ Wv.T computed as [S, E] via lhsT=xT, rhs=wvT
  rope on QT/KT in [e, s] layout (partition-sliced DVE ops from PSUM)
  scoresT[k, q] = lhsT(KT slice).T @ rhs(QT slice)  -- 2 heads row-tiled (K=64)
  expT = exp(scoresT/8 + c): ACT table exp for most tiles; for kt in
         {13,15} of blocks 1..15 a Schraudolph bit-trick on DVE
         (int16(x*1024/ln2 + const) reinterpreted as fp16, ~1.8% RMS)
  ctxT[hd, q] += V_slice.T @ expT  -- 2 heads col-tiled; denom via ones-matmuls

Schedule (v3): single flat software-pipelined loop over all 256 (block, kt)
iterations; scores run one iteration ahead of ctx; V projection interleaved
into block 0; Q/K projections for later pairs spread as 2 filler chunks per
block; ctx+denominator staged once per (head, q-chunk) and DMA'd as one
[65, 512] transfer.

v4: (a) head-dim rows permuted (ROPE_PERM) so rotate-half is an
intra-quadrant 16-row swap -> rope = stream_shuffle + 2 muls + add on the
fp16 2x DVE path instead of 4 partition-sliced psum muls; (b) prologue cut
to K0+Q0 with ACT psum->fp16 assists, K1-3/Q1 moved into block 0 as
fillers; (c) V-tile psum casts moved to ACT; (d) more Schraudolph slots
(3/16 mid, 6/16 in tail blocks where DVE idles).
"""

import os
import sys
import types

import numpy as np

import concourse.bass as bass
import concourse.tile as tile
from concourse import mybir
from concourse.vector_clock import ScopedClock

B, S, D, H = 4, 2048, 1024, 16
HD = 64          # head dim
E = 512          # output dims per core (8 heads)
N_CORES = 8
QC = 512         # q chunk (moving free dim)
KT_TILE = 128    # k tile (stationary cols / psum partitions)
N_QC = S // QC           # 4
N_KT = S // KT_TILE      # 16
FP16 = mybir.dt.float16
FP32 = mybir.dt.float32

# Schraudolph exp-in-fp16-bits: bits = x*(1024/ln2) + (15*1024 + sigma), with
# x = scores*0.125 - 2 folded in: bits = scores*SCH_A + SCH_B, sigma = -59.
SCH_A = 184.6648378
# B = 15360 - 59(sigma) + 453: value ~= exp(s/8)*2^(453/1024) = exp(s/8+0.30663),
# keeping int16 bits positive for scores down to -85 (data min is -76).
SCH_B = 15360.0 - 59.0 + 453.0
ACT_BIAS = 0.3066336   # ACT exp bias matching the Schraudolph 2^(453/1024) scale
# kt slots per block computed on DVE (Schraudolph), tiered by how much PE
# filler work each block range has (less filler -> ACT binds -> more DVE exp)
SCHRAU_KT = (7, 13, 15)                          # blocks 1-9
SCHRAU_KT_MID2 = (3, 5, 7, 9, 11, 13, 15)       # blocks 10-11
SCHRAU_KT_TAIL = (1, 3, 5, 7, 9, 11, 13, 15)    # blocks 12-15
MID2_BLK = 10
TAIL_BLK = 12

# Head-dim row layout: each head's 64 dims stored as [0..15, 32..47] in its
# first 32-partition quadrant and [16..31, 48..63] in its second, so
# rotate-half (i <-> i+32) becomes an intra-quadrant 16-row swap that DVE
# stream_shuffle can do in one pass. Scores are invariant to any shared
# row permutation of Q/K/cos/sin.
ROPE_PERM = ([i for i in range(16)] + [32 + i for i in range(16)]
             + [16 + i for i in range(16)] + [48 + i for i in range(16)])
SHUF_MASK = [i + 16 if i < 16 else i - 16 for i in range(32)]

# ---------------------------------------------------------------------------
# Environment fixups (old nix walrus: max 1 sync wait per instruction; and the
# axon NTFF profile hook module is missing from the image's antenv).
# ---------------------------------------------------------------------------

_PATCHED = False


def _patched_drain_and_barrier(self, tick_clock, wait_clock):
    nc = self.nc
    nops = []
    for _ in range(24):
        nop = mybir.InstNoOp(
            name=nc.get_next_instruction_name(),
            text_hint="wait_split",
            bass_nofuse=True,
            engine=mybir.EngineType.SP,
        )
        nc.add_instruction(nop)
        nops.append(nop)
    drain_inst = nc.sync.drain()
    wait_clock.add_sem_waits(
        drain_inst.ins, ScopedClock({None: tick_clock.global_clock})
    )
    si = drain_inst.ins.sync_info
    if si is not None and si.on_wait and len(si.on_wait) > 1:
        extras = list(si.on_wait[1:])
        si.on_wait = si.on_wait[:1]
        assert len(extras) <= len(nops)
        for nop, w in zip(nops, extras):
            nop.sync_info = mybir.SyncInfo(on_wait=[w], on_update=[])

    nc.all_engine_barrier()
    assert self.sems is not None
    popped = nc._tile_sem_poison_stack.pop()
    assert popped is self._sem_poison
    nc.clear_and_free_semaphores(list(self.sems.allocated().values()))
    nc.all_engine_barrier()


_ORIG_POSTORDER = tile.postorder_instruction_blocks
_SPLIT_COUNTER = [0]


def _split_excess_waits(instructions):
    """Old walrus encodes at most 1 sync wait per instruction (2 for
    EventSemaphore). Hoist extras onto preceding same-engine NoOps — the
    engine is in-order, so gating semantics are identical."""
    for bb_name, insts in instructions.items():
        out = []
        for inst in insts:
            si = getattr(inst, "sync_info", None)
            waits = list(si.on_wait) if (si is not None and si.on_wait) else []
            cap = 2 if isinstance(inst, mybir.InstEventSemaphore) else 1
            if len(waits) > cap:
                eng = inst.engine
                assert eng != mybir.EngineType.Unassigned, (
                    f"multi-wait inst {inst.name} has no engine"
                )
                si.on_wait = waits[:cap]
                for w in waits[cap:]:
                    _SPLIT_COUNTER[0] += 1
                    nop = mybir.InstNoOp(
                        name=f"waitsplit_{_SPLIT_COUNTER[0]}",
                        text_hint="wait_split",
                        bass_nofuse=True,
                        engine=eng,
                        sync_info=mybir.SyncInfo(on_wait=[w], on_update=[]),
                    )
                    out.append(nop)
            out.append(inst)
        instructions[bb_name] = out


def _patched_postorder(instructions, start_bb, output):
    if not output:  # only at the top-level invocation
        _split_excess_waits(instructions)
    return _ORIG_POSTORDER(instructions, start_bb, output)


def _install_fixups():
    global _PATCHED
    if not _PATCHED:
        tile.TileContext._drain_and_barrier = _patched_drain_and_barrier
        tile.postorder_instruction_blocks = _patched_postorder
        _PATCHED = True
    if "antenv.axon_hooks" not in sys.modules:
        mod = types.ModuleType("antenv.axon_hooks")
        _state = {"hook": None}
        mod.set_axon_ntff_profile_hook = lambda h: _state.__setitem__("hook", h)
        mod.get_axon_ntff_profile_hook = lambda: _state["hook"]
        sys.modules["antenv.axon_hooks"] = mod
        try:
            from trn_agent_boot.trn_boot import _ntff_profile_via_ctypes

            mod.set_axon_ntff_profile_hook(
                _ntff_profile_via_ctypes("/opt/axon/libaxon_pjrt.so")
            )
        except Exception:
            pass


# ---------------------------------------------------------------------------
# Kernel build
# ---------------------------------------------------------------------------


def build_nc():
    _install_fixups()
    nc = bass.Bass(trn_type="TRN2", target_bir_lowering=False, debug=False)

    xt_d = nc.dram_tensor("xt", [D, S], FP16, kind="ExternalInput").ap()
    wqt_d = nc.dram_tensor("wqt", [D, E], FP16, kind="ExternalInput").ap()
    wkt_d = nc.dram_tensor("wkt", [D, E], FP16, kind="ExternalInput").ap()
    wvt_d = nc.dram_tensor("wvt", [D, E], FP16, kind="ExternalInput").ap()
    cos2_d = nc.dram_tensor("cos2", [128, S], FP16, kind="ExternalInput").ap()
    sinn2_d = nc.dram_tensor("sinn2", [128, S], FP16, kind="ExternalInput").ap()
    # merged ctx+denominator output: [head, 64 ctx rows + 1 denom row, S]
    o65_d = nc.dram_tensor("o65_out", [8, N_QC, 128, 4 * 65], FP32,
                           kind="ExternalOutput").ap()

    with tile.TileContext(nc) as tc:
        import contextlib

        ctx = contextlib.ExitStack()
        with ctx:
            p_xt = ctx.enter_context(tc.tile_pool(name="xt", bufs=8))
            p_w = ctx.enter_context(tc.tile_pool(name="w", bufs=24))
            p_trig = ctx.enter_context(tc.tile_pool(name="trig", bufs=2))
            p_qk = ctx.enter_context(tc.tile_pool(name="qk", bufs=8))
            p_v = ctx.enter_context(tc.tile_pool(name="v", bufs=16))
            p_exp = ctx.enter_context(tc.tile_pool(name="exp", bufs=6))
            p_tmp = ctx.enter_context(tc.tile_pool(name="tmp", bufs=8))
            p_one = ctx.enter_context(tc.tile_pool(name="one", bufs=1))
            p_stage = ctx.enter_context(tc.tile_pool(name="stage", bufs=4))
            ps_sc = ctx.enter_context(
                tc.tile_pool(name="ps_sc", bufs=2, space="PSUM"))
            ps_cx = ctx.enter_context(
                tc.tile_pool(name="ps_cx", bufs=2, space="PSUM"))
            ps_f = ctx.enter_context(
                tc.tile_pool(name="ps_f", bufs=2, space="PSUM"))

            # ---- loads ----
            # xt on sync HWDGE; wk+wv on gpsimd SWDGE; trig+wq on ACT HWDGE
            # (ordered by first use: proj0 needs wk/trig/wq early, wv later).
            xt_tiles = []
            for dt_i in range(8):
                t = p_xt.tile([128, S], FP16, tag="xt")
                nc.sync.dma_start(t[:], xt_d[dt_i * 128:(dt_i + 1) * 128, :])
                xt_tiles.append(t)

            w_tiles = {}

            def load_w(nm, dram, eng):
                tl = []
                for dt_i in range(8):
                    t = p_w.tile([128, E], FP16, tag="w", name=f"w{nm}{dt_i}")
                    eng.dma_start(t[:], dram[dt_i * 128:(dt_i + 1) * 128, :])
                    tl.append(t)
                w_tiles[nm] = tl

            # weights on gpsimd SWDGE (spreads transfers across DMA rings);
            # trig on the scalar HWDGE queue so it loads in parallel with wk
            load_w("k", wkt_d, nc.gpsimd)
            cos2 = p_trig.tile([128, S], FP16, tag="trig")
            nc.scalar.dma_start(cos2[:], cos2_d[:])
            sinn2 = p_trig.tile([128, S], FP16, tag="trig")
            nc.scalar.dma_start(sinn2[:], sinn2_d[:])
            load_w("q", wqt_d, nc.gpsimd)
            load_w("v", wvt_d, nc.gpsimd)
            expbias = p_one.tile([128, 1], FP32)
            nc.gpsimd.memset(expbias[:], ACT_BIAS)

            qt_tiles = [None] * 4
            kt_tiles = [None] * 4
            v_tiles = [None] * 16

            def proj_start(p, kind, sc):
                """Allocate the psum chunk for proj (p, kind, sc)."""
                if kind == "k":
                    if kt_tiles[p] is None:
                        kt_tiles[p] = p_qk.tile([128, S], FP16, tag="qk",
                                                name=f"ktt{p}")
                    out_tile = kt_tiles[p]
                else:
                    if qt_tiles[p] is None:
                        qt_tiles[p] = p_qk.tile([128, S], FP16, tag="qk",
                                                name=f"qtt{p}")
                    out_tile = qt_tiles[p]
                qp = ps_f.tile([128, QC], FP32, tag="psf",
                               name=f"qp_{kind}{p}_{sc}")
                return (p, kind, sc, qp, out_tile)

            def proj_mm(st8, dt_lo, dt_hi):
                p, kind, sc, qp, _ = st8
                wt = w_tiles[kind]
                for dt_i in range(dt_lo, dt_hi):
                    nc.tensor.matmul(
                        qp[:],
                        lhsT=wt[dt_i][:, p * 128:(p + 1) * 128],
                        rhs=xt_tiles[dt_i][:, sc * QC:(sc + 1) * QC],
                        start=(dt_i == 0),
                        stop=(dt_i == 7),
                    )

            def proj_rope(st8, act_assist=False):
                # head dims are row-permuted (ROPE_PERM) so rotate-half is an
                # intra-quadrant 16-row swap: one DVE stream_shuffle replaces
                # the 4 partition-sliced muls.
                p, kind, sc, qp, out_tile = st8
                cs = slice(sc * QC, (sc + 1) * QC)
                if act_assist:
                    # prologue: ACT (idle) casts psum->fp16 so the DVE ops
                    # run on the 2x 16-bit path and the psum buf frees early
                    src = p_tmp.tile([128, QC], FP16)
                    nc.scalar.copy(src[:], qp[:])
                    qsw = p_tmp.tile([128, QC], FP16)
                else:
                    # shuffle can't cast, so psum fp32 stays fp32 here
                    src = qp
                    qsw = p_tmp.tile([128, QC], FP32)
                nc.vector.stream_shuffle(qsw[:], src[:], SHUF_MASK)
                tmp = p_tmp.tile([128, QC], FP16)
                nc.vector.tensor_mul(tmp[:], qsw[:], sinn2[:, cs])
                tmp2 = p_tmp.tile([128, QC], FP16)
                nc.vector.tensor_mul(tmp2[:], src[:], cos2[:, cs])
                nc.vector.tensor_add(out_tile[:, cs], tmp[:], tmp2[:])

            def emit_proj_chunk(p, kind, sc, act_assist=False):
                st8 = proj_start(p, kind, sc)
                proj_mm(st8, 0, 8)
                proj_rope(st8, act_assist)

            def emit_v_chunk(st):
                vp = ps_f.tile([128, E], FP32, tag="psf", name=f"vp{st}")
                for dt_i in range(8):
                    nc.tensor.matmul(
                        vp[:],
                        lhsT=xt_tiles[dt_i][:, st * 128:(st + 1) * 128],
                        rhs=w_tiles["v"][dt_i][:],
                        start=(dt_i == 0),
                        stop=(dt_i == 7),
                    )
                vt = p_v.tile([128, 8 * 65], FP16, tag="v", name=f"vt{st}")
                vt_r = vt.rearrange("p (h c) -> p h c", h=8)
                nc.vector.memset(vt_r[:, :, 64:65], 1.0)
                # psum->fp16 cast on ACT (block 0's DVE is rope-loaded)
                nc.scalar.copy(
                    vt_r[:, :, 0:64],
                    vp.rearrange("p (h c) -> p h c", h=8),
                )
                v_tiles[st] = vt

            # pair-0 prologue: only K sc0 + Q qc0 before the main loop
            # (unblocks scores kt 0-3); K sc1-3 and Q qc1 move into block 0
            # as fillers so the PE isn't gated on 5 serial DVE ropes.
            emit_proj_chunk(0, "k", 0, act_assist=True)
            emit_proj_chunk(0, "q", 0, act_assist=True)

            # proj filler chunks at (block, kt) slots.
            # Block 0 completes pair 0 inline (ACT-assisted ropes); all
            # remaining chunks run as active fillers (2 MMs/g) in blocks
            # 1-9 only, so blocks 10-15 stay rope-free and the DVE can
            # carry exp halves there without queue delays.
            filler = {}
            filler[(0, 0)] = (0, "k", 1)
            filler[(0, 4)] = (0, "k", 2)
            filler[(0, 8)] = (0, "k", 3)
            filler[(0, 12)] = (0, "q", 1)
            filler[(0, 14)] = (1, "k", 0)
            _SEQ = [(0, "q", 2), (0, "q", 3),
                    (1, "k", 1), (1, "k", 2), (1, "k", 3), (1, "q", 0),
                    (1, "q", 1), (1, "q", 2), (1, "q", 3),
                    (2, "k", 0), (2, "k", 1), (2, "k", 2), (2, "k", 3),
                    (2, "q", 0), (2, "q", 1), (2, "q", 2), (2, "q", 3),
                    (3, "k", 0), (3, "k", 1), (3, "k", 2), (3, "k", 3),
                    (3, "q", 0), (3, "q", 1), (3, "q", 2), (3, "q", 3)]
            _SLOTS = [(b, kt) for b in range(1, 10) for kt in (0, 5, 10)]
            for _sl, _ch in zip(_SLOTS, _SEQ):
                filler[_sl] = _ch

            def emit_scores(g, name):
                blk, kt = g // N_KT, g % N_KT
                p, qc = blk // N_QC, blk % N_QC
                qs = slice(qc * QC, (qc + 1) * QC)
                ks = slice(kt * KT_TILE, (kt + 1) * KT_TILE)
                sc_ps = ps_sc.tile([128, 2 * QC], FP32, tag="pssc", name=name)
                for j in range(2):
                    rs = slice(j * 64, (j + 1) * 64)
                    nc.tensor.matmul(
                        sc_ps[:, j * QC:(j + 1) * QC],
                        lhsT=kt_tiles[p][rs, ks],
                        rhs=qt_tiles[p][rs, qs],
                        start=True,
                        stop=True,
                    )
                return sc_ps

            schrau = {g for g in range(256)
                      if (1 <= g // N_KT < MID2_BLK
                          and g % N_KT in SCHRAU_KT)
                      or (MID2_BLK <= g // N_KT < TAIL_BLK
                          and g % N_KT in SCHRAU_KT_MID2)
                      or (g // N_KT >= TAIL_BLK
                          and g % N_KT in SCHRAU_KT_TAIL)}

            sc_tiles = {0: emit_scores(0, "scps0")}
            emit_v_chunk(0)
            emit_v_chunk(1)

            ex_pre = {}
            ctx_ps = None
            active_fillers = []
            for g in range(256):
                blk, kt = g // N_KT, g % N_KT
                p, qc = blk // N_QC, blk % N_QC
                qs = slice(qc * QC, (qc + 1) * QC)
                if kt == 0:
                    # full-bank tiles; only the (m=0, kt=0) matmul uses
                    # start=True -- it clears the whole bank (hw semantics),
                    # so the other 3 packed chains must accumulate onto it
                    ctx_ps = [
                        ps_cx.tile([128, QC], FP32, tag="pscx",
                                   name=f"ctxps{blk}_{_j}")
                        for _j in range(2)
                    ]
                # exp tile for this iteration (pre-emitted on DVE for
                # Schraudolph slots, else ACT table exp now)
                if g in ex_pre:
                    ex = ex_pre.pop(g)
                else:
                    ex = p_exp.tile([128, 2 * QC], FP16, tag="exp")
                    scg = sc_tiles.pop(g)
                    if blk >= TAIL_BLK:
                        # tail: per-head halves let ctx_h0 start after half
                        # the exp latency
                        for j in range(2):
                            hs = slice(j * QC, (j + 1) * QC)
                            nc.scalar.activation(
                                ex[:, hs], scg[:, hs],
                                mybir.ActivationFunctionType.Exp,
                                scale=0.125, bias=expbias[:],
                            )
                    else:
                        nc.scalar.activation(
                            ex[:], scg[:],
                            mybir.ActivationFunctionType.Exp,
                            scale=0.125, bias=expbias[:],
                        )
                # scores one iteration ahead
                if g + 1 < 256:
                    sc_tiles[g + 1] = emit_scores(g + 1, f"scps{g + 1}")
                    if g + 1 in schrau:
                        ex2 = p_exp.tile([128, 2 * QC], FP16, tag="exp")
                        nc.vector.tensor_scalar(
                            ex2[:].bitcast(mybir.dt.int16),
                            sc_tiles.pop(g + 1)[:],
                            SCH_A, SCH_B,
                            mybir.AluOpType.mult, mybir.AluOpType.add,
                        )
                        ex_pre[g + 1] = ex2
                # V projection rides inside block 0 (all 16 chunks needed
                # by its ctx accumulation; two pre-emitted before the loop)
                if blk == 0 and kt <= 13:
                    emit_v_chunk(kt + 2)
                if (blk, kt) in filler:
                    if blk == 0:
                        emit_proj_chunk(*filler[(blk, kt)], act_assist=True)
                    else:
                        active_fillers.append(
                            [proj_start(*filler[(blk, kt)]), 0])
                for af in active_fillers:
                    st8, done = af
                    proj_mm(st8, done, done + 2)
                    af[1] += 2
                    if af[1] == 8:
                        proj_rope(st8)
                active_fillers = [af for af in active_fillers if af[1] < 8]
                for j in range(2):
                    h_loc = 2 * p + j
                    for m in range(4):
                        nc.tensor.matmul(
                            ctx_ps[j][:, m * 65:(m + 1) * 65],
                            lhsT=ex[:, j * QC + m * 128:j * QC + (m + 1) * 128],
                            rhs=v_tiles[kt][:, h_loc * 65:(h_loc + 1) * 65],
                            start=(kt == 0 and m == 0),
                            stop=(kt == N_KT - 1),
                            skip_group_check=True,
                        )
                if kt == N_KT - 1:
                    for j in range(2):
                        h_loc = 2 * p + j
                        st = p_stage.tile([128, 4 * 65], FP32, tag="stage")
                        nc.vector.tensor_copy(st[:], ctx_ps[j][:, 0:260])
                        nc.sync.dma_start(o65_d[h_loc, qc], st[:])
    return nc


_NC_CACHE = None


def _get_nc():
    global _NC_CACHE
    if _NC_CACHE is None:
        _NC_CACHE = build_nc()
    return _NC_CACHE


# ---------------------------------------------------------------------------
# Host-side shard / gather
# ---------------------------------------------------------------------------

LAST_EXEC_TIME_NS = None


def kernel(hidden_states, attention_mask, rope_cos, rope_sin, Wq, Wk, Wv):
    """Full inputs -> full output [4, 2048, 1024] float32."""
    global LAST_EXEC_TIME_NS
    del attention_mask  # module sets it to None in forward

    from concourse.bass_utils import run_bass_kernel_spmd

    hidden_states = np.asarray(hidden_states)
    rope_cos = np.asarray(rope_cos, dtype=np.float32)
    rope_sin = np.asarray(rope_sin, dtype=np.float32)
    Wq, Wk, Wv = (np.asarray(w) for w in (Wq, Wk, Wv))

    # trig tables in [e, s] layout, duplicated across the 2 heads of a tile,
    # sin sign-folded for rotate-half, rows permuted by ROPE_PERM; fp16 for
    # 2x DVE rope
    perm = np.asarray(ROPE_PERM)
    cosT = rope_cos.T.astype(np.float32)[perm]      # [64, S]
    sinT = rope_sin.T.astype(np.float32)
    sinN = np.concatenate([-sinT[:32], sinT[32:]], axis=0)[perm]  # [64, S]
    cos2 = np.ascontiguousarray(
        np.concatenate([cosT, cosT], axis=0)).astype(np.float16)   # [128,S]
    sinn2 = np.ascontiguousarray(
        np.concatenate([sinN, sinN], axis=0)).astype(np.float16)   # [128,S]
    # Wq/Wk output dims permuted per head to match (V / output unpermuted)
    perm_e = np.concatenate([h * 64 + perm for h in range(H)])     # [D]

    in_maps = []
    for c in range(N_CORES):
        b, g = c // 2, c % 2
        xT = np.ascontiguousarray(hidden_states[b].T).astype(np.float16)
        sl = slice(g * E, (g + 1) * E)
        Wq_p = Wq[perm_e][sl, :]
        Wk_p = Wk[perm_e][sl, :]
        in_maps.append({
            "xt": xT,
            "wqt": np.ascontiguousarray(Wq_p.T).astype(np.float16),
            "wkt": np.ascontiguousarray(Wk_p.T).astype(np.float16),
            "wvt": np.ascontiguousarray(Wv[sl, :].T).astype(np.float16),
            "cos2": cos2,
            "sinn2": sinn2,
        })

    nc = _get_nc()
    trace = bool(os.environ.get("BERT_KERNEL_TRACE"))
    res = run_bass_kernel_spmd(nc, in_maps, list(range(N_CORES)), trace=trace)
    LAST_EXEC_TIME_NS = res.exec_time_ns

    out = np.empty((B, S, D), dtype=np.float32)
    for c in range(N_CORES):
        b, g = c // 2, c % 2
        o65 = res.results[c]["o65_out"]   # [8, N_QC, 128, 4*65]
        oq = o65.reshape(8, N_QC, 128, 4, 65).transpose(0, 1, 3, 2, 4)
        oq = np.ascontiguousarray(oq).reshape(8, S, 65)  # q-major per head
        for hh in range(8):
            col = g * E + hh * 64
            out[b, :, col:col + 64] = oq[hh, :, 0:64] / oq[hh, :, 64][:, None]
    return out

